# revision 15
# baseline (speedup 1.0000x reference)
"""Trainium2 Bass kernel for DeformationTrackerBiFlowModel (non-teacher-forcing).

Math (per batch element b, per step t):
    x_t   = [prev_out (2), fin_t (3)]            (5,)
    h_t   = tanh(x_t @ W_rnn + b_rnn)            (12,)   (U_rnn is inert: h0 == 0)
    out_t = [cp0 (2), h_t (12)] @ W_out + b_out  (2,)
    prev_out_{t+1} = out_t;  prev_out_0 = cp0

Device mapping:
  - Pure data parallelism: batch B=65536 sharded over 8 cores (8192 each,
    padded to 8200 = G*C*COLS).
  - Feature-major layout: features on SBUF partitions, batch on the free dim.
    G=10 independent trajectories are packed block-diagonally into the PE
    array per matmul (K=5G=50 -> M=12G=120, then K=12G=120 -> M=2G=20), with
    C=2 independent column-chains of COLS=410 so engines can pipeline across
    the sequential 100-step recurrence.
  - The constant c = cp0 @ W_out[:2] + b_out is precomputed on host; per step
    the device computes out = (h @ W_out[2:]) + c with a single DVE add which
    also writes the next step's matmul input rows (the recurrence).
"""

import os
from contextlib import ExitStack

import numpy as np

import concourse.bass as bass
import concourse.mybir as mybir
import concourse.tile as tile
from concourse import bacc
from concourse.bass_utils import run_bass_kernel_spmd

B, T = 65536, 100
D_CP, D_FIN, HID = 2, 3, 12
NCORES = 8
BC = B // NCORES              # 8192 per core
G = 10                        # trajectories packed per matmul (block-diag)
C = 2                         # independent column chains
COLS = 410                    # batch columns per chain
BP = G * C * COLS             # 8200 padded batch per core

F32 = mybir.dt.float32

# float32r streams 1 col/cycle on the PE (vs 4 for exact fp32) at slightly
# reduced internal precision. Set DTB_MM_F32=1 to force exact fp32 matmuls.
MM_DTYPE = F32 if os.environ.get("DTB_MM_F32") else mybir.dt.float32r

LAST_RESULTS = None  # test.py introspects profiling info from here


def build_program(t_steps=T, g=G, c=C, cols=COLS, mm_dtype=None):
    if mm_dtype is None:
        mm_dtype = MM_DTYPE
    # Everything on the matmul input path (x, h, weights and the DRAM tensors
    # DMA'd into them) carries mm_dtype: with float32r the DVE/ACT writes
    # round, and DMAs are same-dtype bypass, satisfying the BIR verifier's
    # "rounded to FP32r" producer rule with zero extra instructions.
    XDT = mm_dtype
    nc = bacc.Bacc(target_bir_lowering=False)

    # fin carries D_FIN*g rows plus one constant-ones row: b_rnn rides in
    # w1f's last row, so the tanh activation needs no bias operand (keeps
    # it at a single sync wait).
    fin = nc.dram_tensor("fin", [t_steps, c, D_FIN * g + 1, cols], XDT, kind="ExternalInput")
    x0 = nc.dram_tensor("x0", [c, D_CP * g, cols], F32, kind="ExternalInput")
    cb = nc.dram_tensor("cb", [c, D_CP * g, cols], F32, kind="ExternalInput")
    w1p = nc.dram_tensor("w1p", [D_CP * g, HID * g], F32, kind="ExternalInput")
    w1f = nc.dram_tensor("w1f", [D_FIN * g + 1, HID * g], F32, kind="ExternalInput")
    w2 = nc.dram_tensor("w2", [HID * g, D_CP * g], F32, kind="ExternalInput")
    out = nc.dram_tensor("out", [t_steps, c, D_CP * g, cols], XDT, kind="ExternalOutput")

    tanh = mybir.ActivationFunctionType.Tanh

    with tile.TileContext(nc) as tc, ExitStack() as ctx:
        const = ctx.enter_context(tc.tile_pool(name="const", bufs=1))
        xppool = ctx.enter_context(tc.tile_pool(name="xppool", bufs=4))
        xfpool = ctx.enter_context(tc.tile_pool(name="xfpool", bufs=4))
        hpool = ctx.enter_context(tc.tile_pool(name="hpool", bufs=2))
        psum = ctx.enter_context(tc.tile_pool(name="psum", bufs=2, space="PSUM"))

        # Constants are staged DMA -> tmp, then copied by the DVE: every
        # matmul input is produced by exactly one compute engine, keeping each
        # fp32/f32r (self-loading) matmul within its single sync-wait slot.
        # The DVE copy also rounds to float32r when mm_dtype is float32r.
        w1at = const.tile([D_CP * g, HID * g], F32, name="w1at")
        nc.sync.dma_start(out=w1at, in_=w1p[:, :])
        w1a = const.tile([D_CP * g, HID * g], XDT, name="w1a")
        nc.vector.tensor_copy(w1a, w1at)
        w1bt = const.tile([D_FIN * g + 1, HID * g], F32, name="w1bt")
        nc.sync.dma_start(out=w1bt, in_=w1f[:, :])
        w1b = const.tile([D_FIN * g + 1, HID * g], XDT, name="w1b")
        nc.vector.tensor_copy(w1b, w1bt)
        w2t = const.tile([HID * g, D_CP * g], F32, name="w2t")
        nc.sync.dma_start(out=w2t, in_=w2[:, :])
        w2s = const.tile([HID * g, D_CP * g], XDT, name="w2s")
        nc.vector.tensor_copy(w2s, w2t)
        cbs = []
        for ch in range(c):
            cbt = const.tile([D_CP * g, cols], F32, tag=f"cbt{ch}", name=f"cbt{ch}")
            nc.sync.dma_start(out=cbt, in_=cb[ch])
            cbc = const.tile([D_CP * g, cols], F32, tag=f"cb{ch}", name=f"cbs{ch}")
            nc.vector.tensor_copy(cbc, cbt)
            cbs.append(cbc)

        xprevs, xfins = [], []
        for ch in range(c):
            x0t = const.tile([D_CP * g, cols], F32, tag=f"x0t{ch}", name=f"x0t{ch}")
            nc.sync.dma_start(out=x0t, in_=x0[ch])
            xp = xppool.tile([D_CP * g, cols], XDT, tag=f"xp{ch}", name=f"xp_{ch}_0")
            nc.vector.tensor_copy(xp, x0t)
            xf = xfpool.tile([D_FIN * g + 1, cols], XDT, tag=f"xf{ch}", name=f"xf_{ch}_0")
            nc.sync.dma_start(out=xf, in_=fin[0, ch])
            xprevs.append(xp)
            xfins.append(xf)

        for t in range(t_steps):
            for ch in range(c):
                p1 = psum.tile([HID * g, cols], F32, tag=f"p1{ch}", name=f"p1_{ch}_{t}")
                nc.tensor.matmul(p1, w1a, xprevs[ch], start=True, stop=False)
                nc.tensor.matmul(p1, w1b, xfins[ch], start=False, stop=True)
                h = hpool.tile([HID * g, cols], XDT, tag=f"h{ch}", name=f"h_{ch}_{t}")
                nc.scalar.activation(h, p1, tanh)
                p2 = psum.tile([D_CP * g, cols], F32, tag=f"p2{ch}", name=f"p2_{ch}_{t}")
                nc.tensor.matmul(p2, w2s, h, start=True, stop=True)
                if t + 1 < t_steps:
                    xf = xfpool.tile([D_FIN * g + 1, cols], XDT, tag=f"xf{ch}", name=f"xf_{ch}_{t + 1}")
                    nc.sync.dma_start(out=xf, in_=fin[t + 1, ch])
                    xfins[ch] = xf
                xp = xppool.tile([D_CP * g, cols], XDT, tag=f"xp{ch}", name=f"xp_{ch}_{t + 1}")
                nc.vector.tensor_add(xp, p2, cbs[ch])
                nc.sync.dma_start(out=out[t, ch], in_=xp)
                xprevs[ch] = xp
    nc.compile()
    return nc


def build_packed_weights(W_rnn, W_out, b_rnn, g=G):
    w1p = np.zeros((D_CP * g, HID * g), np.float32)
    w1f = np.zeros((D_FIN * g + 1, HID * g), np.float32)
    w2 = np.zeros((HID * g, D_CP * g), np.float32)
    for i in range(g):
        w1p[D_CP * i : D_CP * (i + 1), HID * i : HID * (i + 1)] = W_rnn[:D_CP]
        w1f[D_FIN * i : D_FIN * (i + 1), HID * i : HID * (i + 1)] = W_rnn[D_CP:]
        w1f[D_FIN * g, HID * i : HID * (i + 1)] = np.asarray(b_rnn, np.float32)
        w2[HID * i : HID * (i + 1), D_CP * i : D_CP * (i + 1)] = W_out[D_CP:]
    return w1p, w1f, w2


def stage_inputs(cp0, fin, cvec, g=G, c=C, cols=COLS, t_steps=T):
    """Per-core staging: (BC,...) batch-major -> feature-major device layouts."""
    bp = g * c * cols
    bc = cp0.shape[0]
    fin_p = np.zeros((bp, t_steps, D_FIN), np.float32)
    fin_p[:bc] = fin
    cp0_p = np.zeros((bp, D_CP), np.float32)
    cp0_p[:bc] = cp0
    cv_p = np.zeros((bp, D_CP), np.float32)
    cv_p[:bc] = cvec
    # b = ch*(g*cols) + gi*cols + j
    fin_d = np.ones((t_steps, c, D_FIN * g + 1, cols), np.float32)
    fin_d[:, :, : D_FIN * g, :] = fin_p.reshape(c, g, cols, t_steps, D_FIN).transpose(
        3, 0, 1, 4, 2
    ).reshape(t_steps, c, D_FIN * g, cols)
    x0_d = np.ascontiguousarray(
        cp0_p.reshape(c, g, cols, D_CP).transpose(0, 1, 3, 2)
    ).reshape(c, D_CP * g, cols)
    cb_d = np.ascontiguousarray(
        cv_p.reshape(c, g, cols, D_CP).transpose(0, 1, 3, 2)
    ).reshape(c, D_CP * g, cols)
    return fin_d, x0_d, cb_d


def unstage_output(out_d, bc, g=G, c=C, cols=COLS, t_steps=T):
    """(T, C, 2G, COLS) device layout -> (bc, T, 2) batch-major."""
    bp = g * c * cols
    o = out_d.reshape(t_steps, c, g, D_CP, cols).transpose(1, 2, 4, 0, 3)
    return np.ascontiguousarray(o).reshape(bp, t_steps, D_CP)[:bc]


def kernel(control_point_input, finger_input, W_rnn, U_rnn, b_rnn, W_out, b_out):
    global LAST_RESULTS
    cp = np.asarray(control_point_input, np.float32)
    fin = np.asarray(finger_input, np.float32)
    W_rnn = np.asarray(W_rnn, np.float32)
    b_rnn = np.asarray(b_rnn, np.float32)
    W_out = np.asarray(W_out, np.float32)
    b_out = np.asarray(b_out, np.float32)

    cp0 = cp[:, 0, :]                                  # (B, 2)
    cvec = cp0 @ W_out[:D_CP] + b_out                  # (B, 2), constant per step
    w1p, w1f, w2 = build_packed_weights(W_rnn, W_out, b_rnn)

    nc = build_program()
    in_maps = []
    for m in range(NCORES):
        sl = slice(m * BC, (m + 1) * BC)
        fin_d, x0_d, cb_d = stage_inputs(cp0[sl], fin[sl], cvec[sl])
        in_maps.append(
            {"fin": fin_d, "x0": x0_d, "cb": cb_d, "w1p": w1p, "w1f": w1f,
             "w2": w2}
        )

    trace = bool(os.environ.get("DTB_TRACE"))
    res = run_bass_kernel_spmd(
        nc, in_maps, core_ids=list(range(NCORES)), trace=trace
    )
    LAST_RESULTS = res

    outs = [unstage_output(res.results[m]["out"], BC) for m in range(NCORES)]
    return np.concatenate(outs, axis=0)


# revision 16
# speedup vs baseline: 1.3652x; 1.3652x over previous
"""Trainium2 Bass kernel for DeformationTrackerBiFlowModel (non-teacher-forcing).

Math (per batch element b, per step t):
    x_t   = [prev_out (2), fin_t (3)]            (5,)
    h_t   = tanh(x_t @ W_rnn + b_rnn)            (12,)   (U_rnn is inert: h0 == 0)
    out_t = [cp0 (2), h_t (12)] @ W_out + b_out  (2,)
    prev_out_{t+1} = out_t;  prev_out_0 = cp0

Device mapping:
  - Pure data parallelism: batch B=65536 sharded over 8 cores (8192 each,
    padded to 8200 = G*C*COLS).
  - Feature-major layout: features on SBUF partitions, batch on the free dim.
    G=10 independent trajectories are packed block-diagonally into the PE
    array per matmul (K=5G=50 -> M=12G=120, then K=12G=120 -> M=2G=20), with
    C=2 independent column-chains of COLS=410 so engines can pipeline across
    the sequential 100-step recurrence.
  - The constant c = cp0 @ W_out[:2] + b_out is precomputed on host; per step
    the device computes out = (h @ W_out[2:]) + c with a single DVE add which
    also writes the next step's matmul input rows (the recurrence).
"""

import os
from contextlib import ExitStack

import numpy as np

import concourse.bass as bass
import concourse.mybir as mybir
import concourse.tile as tile
from concourse import bacc
from concourse.bass_utils import run_bass_kernel_spmd

B, T = 65536, 100
D_CP, D_FIN, HID = 2, 3, 12
NCORES = 8
BC = B // NCORES              # 8192 per core
G = 10                        # trajectories packed per matmul (block-diag)
C = 2                         # independent column chains
COLS = 410                    # batch columns per chain
BP = G * C * COLS             # 8200 padded batch per core

F32 = mybir.dt.float32

# Matmul-path dtype. bf16 streams 1 col/cycle on the PE with HAM warm-up;
# float32r measured ~1.6-2 cycles/col; exact fp32 is 4 cycles/col.
# Select with DTB_MM in {bf16, f32r, f32}; default bf16.
_MM_CHOICES = {"bf16": mybir.dt.bfloat16, "f32r": mybir.dt.float32r, "f32": F32}
MM_DTYPE = _MM_CHOICES[os.environ.get("DTB_MM", "bf16")]

MM_NP = mybir.dt.np(MM_DTYPE)  # numpy dtype of fin/out device tensors

LAST_RESULTS = None  # test.py introspects profiling info from here


def build_program(t_steps=T, g=G, c=C, cols=COLS, mm_dtype=None):
    if mm_dtype is None:
        mm_dtype = MM_DTYPE
    # Everything on the matmul input path (x, h, weights and the DRAM tensors
    # DMA'd into them) carries mm_dtype: with float32r the DVE/ACT writes
    # round, and DMAs are same-dtype bypass, satisfying the BIR verifier's
    # "rounded to FP32r" producer rule with zero extra instructions.
    XDT = mm_dtype
    nc = bacc.Bacc(target_bir_lowering=False)

    # fin carries D_FIN*g rows plus one constant-ones row: b_rnn rides in
    # w1f's last row, so the tanh activation needs no bias operand (keeps
    # it at a single sync wait).
    fin = nc.dram_tensor("fin", [t_steps, c, D_FIN * g + 1, cols], XDT, kind="ExternalInput")
    x0 = nc.dram_tensor("x0", [c, 5 * g + 1, cols], F32, kind="ExternalInput")
    cb = nc.dram_tensor("cb", [c, D_CP * g, cols], F32, kind="ExternalInput")
    w1 = nc.dram_tensor("w1", [5 * g + 1, HID * g], F32, kind="ExternalInput")
    w2 = nc.dram_tensor("w2", [HID * g, D_CP * g], F32, kind="ExternalInput")
    out = nc.dram_tensor("out", [t_steps, c, D_CP * g, cols], XDT, kind="ExternalOutput")

    tanh = mybir.ActivationFunctionType.Tanh

    with tile.TileContext(nc) as tc, ExitStack() as ctx:
        const = ctx.enter_context(tc.tile_pool(name="const", bufs=1))
        xppool = ctx.enter_context(tc.tile_pool(name="xppool", bufs=4))
        xfpool = ctx.enter_context(tc.tile_pool(name="xfpool", bufs=4))
        hpool = ctx.enter_context(tc.tile_pool(name="hpool", bufs=2))
        psum = ctx.enter_context(tc.tile_pool(name="psum", bufs=2, space="PSUM"))

        # Constants are staged DMA -> tmp, then copied by the DVE: every
        # matmul input is produced by exactly one compute engine, keeping each
        # fp32/f32r (self-loading) matmul within its single sync-wait slot.
        # The DVE copy also rounds to float32r when mm_dtype is float32r.
        w1t = const.tile([5 * g + 1, HID * g], F32, name="w1t")
        nc.sync.dma_start(out=w1t, in_=w1[:, :])
        w1s = const.tile([5 * g + 1, HID * g], XDT, name="w1s")
        nc.vector.tensor_copy(w1s, w1t)
        w2t = const.tile([HID * g, D_CP * g], F32, name="w2t")
        nc.sync.dma_start(out=w2t, in_=w2[:, :])
        w2s = const.tile([HID * g, D_CP * g], XDT, name="w2s")
        nc.vector.tensor_copy(w2s, w2t)
        cbs = []
        for ch in range(c):
            cbt = const.tile([D_CP * g, cols], F32, tag=f"cbt{ch}", name=f"cbt{ch}")
            nc.sync.dma_start(out=cbt, in_=cb[ch])
            cbc = const.tile([D_CP * g, cols], F32, tag=f"cb{ch}", name=f"cbs{ch}")
            nc.vector.tensor_copy(cbc, cbt)
            cbs.append(cbc)

        xts = []
        for ch in range(c):
            x0t = const.tile([5 * g + 1, cols], F32, tag=f"x0t{ch}", name=f"x0t{ch}")
            nc.sync.dma_start(out=x0t, in_=x0[ch])
            xt = xppool.tile([5 * g + 1, cols], XDT, tag=f"x{ch}", name=f"x_{ch}_0")
            nc.vector.tensor_copy(xt, x0t)
            xts.append(xt)

        for t in range(t_steps):
            for ch in range(c):
                xt = xts[ch]
                p1 = psum.tile([HID * g, cols], F32, tag=f"p1{ch}", name=f"p1_{ch}_{t}")
                nc.tensor.matmul(p1, w1s, xt, start=True, stop=True)
                h = hpool.tile([HID * g, cols], XDT, tag=f"h{ch}", name=f"h_{ch}_{t}")
                nc.scalar.activation(h, p1, tanh)
                p2 = psum.tile([D_CP * g, cols], F32, tag=f"p2{ch}", name=f"p2_{ch}_{t}")
                nc.tensor.matmul(p2, w2s, h, start=True, stop=True)
                if t + 1 < t_steps:
                    xn = xppool.tile([5 * g + 1, cols], XDT, tag=f"x{ch}", name=f"x_{ch}_{t + 1}")
                    nc.sync.dma_start(out=xn[D_CP * g :, :], in_=fin[t + 1, ch])
                    odst = xn[0 : D_CP * g, :]
                else:
                    ot = hpool.tile([D_CP * g, cols], XDT, tag=f"o{ch}", name=f"o_{ch}_{t}")
                    odst = ot
                nc.vector.tensor_add(odst, p2, cbs[ch])
                nc.sync.dma_start(out=out[t, ch], in_=odst)
                if t + 1 < t_steps:
                    xts[ch] = xn
    nc.compile()
    return nc


def build_packed_weights(W_rnn, W_out, b_rnn, g=G):
    w1 = np.zeros((5 * g + 1, HID * g), np.float32)
    w2 = np.zeros((HID * g, D_CP * g), np.float32)
    for i in range(g):
        w1[D_CP * i : D_CP * (i + 1), HID * i : HID * (i + 1)] = W_rnn[:D_CP]
        w1[D_CP * g + D_FIN * i : D_CP * g + D_FIN * (i + 1), HID * i : HID * (i + 1)] = W_rnn[D_CP:]
        w1[5 * g, HID * i : HID * (i + 1)] = np.asarray(b_rnn, np.float32)
        w2[HID * i : HID * (i + 1), D_CP * i : D_CP * (i + 1)] = W_out[D_CP:]
    return w1, w2


def stage_inputs(cp0, fin, cvec, g=G, c=C, cols=COLS, t_steps=T):
    """Per-core staging: (BC,...) batch-major -> feature-major device layouts."""
    bp = g * c * cols
    bc = cp0.shape[0]
    fin_p = np.zeros((bp, t_steps, D_FIN), np.float32)
    fin_p[:bc] = fin
    cp0_p = np.zeros((bp, D_CP), np.float32)
    cp0_p[:bc] = cp0
    cv_p = np.zeros((bp, D_CP), np.float32)
    cv_p[:bc] = cvec
    # b = ch*(g*cols) + gi*cols + j
    fin_d = np.ones((t_steps, c, D_FIN * g + 1, cols), np.float32)
    fin_d[:, :, : D_FIN * g, :] = fin_p.reshape(c, g, cols, t_steps, D_FIN).transpose(
        3, 0, 1, 4, 2
    ).reshape(t_steps, c, D_FIN * g, cols)
    # x0 = full 51-row first-step input: [cp0 rows; fin_0 rows (incl. ones)]
    x0_d = np.empty((c, 5 * g + 1, cols), np.float32)
    x0_d[:, : D_CP * g] = cp0_p.reshape(c, g, cols, D_CP).transpose(0, 1, 3, 2).reshape(
        c, D_CP * g, cols
    )
    x0_d[:, D_CP * g :] = fin_d[0]
    cb_d = np.ascontiguousarray(
        cv_p.reshape(c, g, cols, D_CP).transpose(0, 1, 3, 2)
    ).reshape(c, D_CP * g, cols)
    return fin_d, x0_d, cb_d


def unstage_output(out_d, bc, g=G, c=C, cols=COLS, t_steps=T):
    """(T, C, 2G, COLS) device layout -> (bc, T, 2) batch-major."""
    bp = g * c * cols
    o = out_d.reshape(t_steps, c, g, D_CP, cols).transpose(1, 2, 4, 0, 3)
    return np.ascontiguousarray(o).reshape(bp, t_steps, D_CP)[:bc]


def kernel(control_point_input, finger_input, W_rnn, U_rnn, b_rnn, W_out, b_out):
    global LAST_RESULTS
    cp = np.asarray(control_point_input, np.float32)
    fin = np.asarray(finger_input, np.float32)
    W_rnn = np.asarray(W_rnn, np.float32)
    b_rnn = np.asarray(b_rnn, np.float32)
    W_out = np.asarray(W_out, np.float32)
    b_out = np.asarray(b_out, np.float32)

    cp0 = cp[:, 0, :]                                  # (B, 2)
    cvec = cp0 @ W_out[:D_CP] + b_out                  # (B, 2), constant per step
    w1, w2 = build_packed_weights(W_rnn, W_out, b_rnn)

    nc = build_program()
    in_maps = []
    for m in range(NCORES):
        sl = slice(m * BC, (m + 1) * BC)
        fin_d, x0_d, cb_d = stage_inputs(cp0[sl], fin[sl], cvec[sl])
        in_maps.append(
            {"fin": fin_d.astype(MM_NP, copy=False), "x0": x0_d, "cb": cb_d,
             "w1": w1, "w2": w2}
        )

    trace = bool(os.environ.get("DTB_TRACE"))
    res = run_bass_kernel_spmd(
        nc, in_maps, core_ids=list(range(NCORES)), trace=trace
    )
    LAST_RESULTS = res

    outs = [
        unstage_output(np.asarray(res.results[m]["out"], np.float32), BC)
        for m in range(NCORES)
    ]
    return np.concatenate(outs, axis=0)
